# revision 1
# baseline (speedup 1.0000x reference)
"""AttnBlockST Trainium2 kernel.

Two SPMD phases on 8 NeuronCores:
  phase 1 (spatial): data-parallel over b*t (32 samples -> 4/core),
    attention over hw=1024 within each (bt, c, hw) sample.
  phase 2 (temporal): data-parallel over b*h*w (2048 -> 256/core),
    attention over t=16, 8 samples packed per 128-partition group with a
    block-diagonal logit mask.

GroupNorm affine (gamma/beta) and the c**-0.5 logit scale are folded into
the conv1x1 weights host-side, so the device only does raw (x-m)*rstd.
Matmuls run in bf16 (fp32 PSUM accumulate); softmax and GN statistics are
fp32.
"""

import numpy as np
import ml_dtypes
from contextlib import ExitStack

import concourse.bass as bass
import concourse.mybir as mybir
import concourse.tile as tile
from concourse.bass_utils import run_bass_kernel_spmd

# ---- walrus workaround: split multi-wait final drain ----
from concourse.vector_clock import ScopedClock
from concourse.tile import TileContext


def _patched_drain_and_barrier(self, tick_clock, wait_clock):
    nc = self.nc
    drain_inst = nc.sync.drain()
    wait_clock.add_sem_waits(
        drain_inst.ins, ScopedClock({None: tick_clock.global_clock})
    )
    si = drain_inst.ins.sync_info
    if si is not None and len(si.on_wait) > 1:
        waits = list(si.on_wait)
        drain_inst.ins.sync_info = mybir.SyncInfo(
            on_wait=waits[:1], on_update=list(si.on_update)
        )
        for w in waits[1:]:
            n = nc.sync.nop(nofuse=True, hint="drain_wait_split")
            n.ins.sync_info = mybir.SyncInfo(on_wait=[w], on_update=[])
    nc.all_engine_barrier()
    assert self.sems is not None
    popped = nc._tile_sem_poison_stack.pop()
    assert popped is self._sem_poison
    nc.clear_and_free_semaphores(list(self.sems.allocated().values()))
    nc.all_engine_barrier()


TileContext._drain_and_barrier = _patched_drain_and_barrier

# ---- problem constants (hardcoded per spec) ----
B, C, T, H, W = 2, 512, 16, 32, 32
GROUPS = 32
EPS = 1e-6
N_CORES = 8
P = 128
CCH = C // P          # 4 channel chunks
GPC = GROUPS // CCH   # 8 groups per 128-channel chunk
GS = C // GROUPS      # 16 channels per group

L1 = H * W            # 1024 spatial positions
NS1 = (B * T) // N_CORES   # 4 samples per core, phase 1
LCH1 = L1 // P        # 8 position chunks

NT2 = 16              # temporal length
NS2 = (B * H * W) // N_CORES  # 256 samples per core, phase 2
HALF = NS2 // 2       # process in halves of 128 samples
F2 = HALF * NT2       # 2048 free columns per half
NB2 = F2 // 512       # 4 n-blocks of 512
NGRP = F2 // P        # 16 groups of 8 samples per half

F32 = mybir.dt.float32
BF16 = mybir.dt.bfloat16
AX = mybir.AxisListType.X
AF = mybir.ActivationFunctionType
OP = None


def _op():
    from concourse.alu_op_type import AluOpType
    return AluOpType


def _bcast_inner(ap, n):
    """View (P, F) access pattern as (P, F, n) with stride-0 inner dim."""
    return bass.AP(tensor=ap.tensor, offset=ap.offset, ap=list(ap.ap) + [[0, n]])


def _split_waits(nc, limit=1):
    """This walrus build rejects >1 sem wait on every ISA template tested
    (LDWEIGHTS, CTRL, ACT, DVE TensorScalar); hoist extra waits onto
    same-engine NoOps placed just before."""
    ctr = [0]
    for f in nc.m.functions:
        for b in f.blocks:
            new = []
            for ins in b.instructions:
                si = getattr(ins, "sync_info", None)
                waits = list(si.on_wait) if si is not None and si.on_wait else []
                lim = limit
                if len(waits) > lim:
                    for w in waits[lim:]:
                        ctr[0] += 1
                        new.append(mybir.InstNoOp(
                            name=f"wsplit-{ctr[0]}",
                            sync_info=mybir.SyncInfo(on_wait=[w], on_update=[]),
                            bass_nofuse=True,
                            engine=ins.engine,
                        ))
                    ins.sync_info = mybir.SyncInfo(
                        on_wait=waits[:lim], on_update=list(si.on_update)
                    )
                new.append(ins)
            b.instructions = new
    return nc


# ---------------------------------------------------------------- phase 1
def build_spatial(reps=1):
    nc = bass.Bass()
    xs = nc.dram_tensor("xs", [NS1, C, L1], F32, kind="ExternalInput")
    ys = nc.dram_tensor("ys", [NS1, C, L1], F32, kind="ExternalOutput")
    wd = {
        n: nc.dram_tensor(n, [C, C], BF16, kind="ExternalInput")
        for n in ("wq", "wk", "wv", "wo")
    }
    bd = {
        n: nc.dram_tensor(n, [P, CCH], F32, kind="ExternalInput")
        for n in ("bq", "bk", "bv", "bo")
    }
    gmask_d = nc.dram_tensor("gmask", [P, GPC], F32, kind="ExternalInput")
    bmask_d = nc.dram_tensor("bmask", [GPC, P], F32, kind="ExternalInput")
    ident_d = nc.dram_tensor("ident", [P, P], BF16, kind="ExternalInput")
    A = _op()

    with tile.TileContext(nc) as tc, ExitStack() as ctx:
        const = ctx.enter_context(tc.tile_pool(name="const", bufs=1))
        stp = ctx.enter_context(tc.tile_pool(name="stats", bufs=3))
        xp = ctx.enter_context(tc.tile_pool(name="x", bufs=2))
        hp = ctx.enter_context(tc.tile_pool(name="h", bufs=2))
        qp = ctx.enter_context(tc.tile_pool(name="q", bufs=2))
        kp = ctx.enter_context(tc.tile_pool(name="k", bufs=2))
        vp = ctx.enter_context(tc.tile_pool(name="v", bufs=2))
        pp = ctx.enter_context(tc.tile_pool(name="pm", bufs=3))
        ptp = ctx.enter_context(tc.tile_pool(name="pt", bufs=2))
        yp = ctx.enter_context(tc.tile_pool(name="y", bufs=3))
        psA = ctx.enter_context(tc.tile_pool(name="psA", bufs=2, space="PSUM"))
        psB = ctx.enter_context(tc.tile_pool(name="psB", bufs=2, space="PSUM"))

        w_sb = {}
        for n in wd:
            t = const.tile([P, CCH, C], BF16, tag=n)
            nc.sync.dma_start(out=t, in_=wd[n].rearrange("(k p) o -> p k o", p=P))
            w_sb[n] = t
        b_sb = {}
        for n in bd:
            t = const.tile([P, CCH], F32, tag=n)
            nc.sync.dma_start(out=t, in_=bd[n][:, :])
            b_sb[n] = t
        gmask = const.tile([P, GPC], F32, tag="gmask")
        nc.sync.dma_start(out=gmask, in_=gmask_d[:, :])
        bmask = const.tile([GPC, P], F32, tag="bmask")
        nc.sync.dma_start(out=bmask, in_=bmask_d[:, :])
        ident = const.tile([P, P], BF16, tag="ident")
        nc.sync.dma_start(out=ident, in_=ident_d[:, :])
        eps_t = const.tile([GPC, 1], F32, tag="eps")
        nc.vector.memset(eps_t, EPS)

        for i_rep in range(reps * NS1):
            i = i_rep % NS1
            x_sb = xp.tile([P, CCH, L1], F32)
            nc.sync.dma_start(out=x_sb, in_=xs[i].rearrange("(k p) l -> p k l", p=P))

            # ---- GroupNorm -> h (bf16) ----
            h_sb = hp.tile([P, CCH, L1], BF16, tag="h")
            for k in range(CCH):
                xc = x_sb[:, k, :]
                st = stp.tile([P, 2, 6], F32, tag="bnst")
                nc.vector.bn_stats(out=st[:, 0, :], in_=xc[:, 0:512])
                nc.vector.bn_stats(out=st[:, 1, :], in_=xc[:, 512:1024])
                mv = stp.tile([P, 2], F32, tag="mv")
                nc.vector.bn_aggr(out=mv, in_=st)
                me = stp.tile([P, 2], F32, tag="me")
                nc.vector.tensor_copy(out=me[:, 0:1], in_=mv[:, 0:1])
                m2 = stp.tile([P, 1], F32, tag="m2")
                nc.vector.tensor_mul(out=m2, in0=mv[:, 0:1], in1=mv[:, 0:1])
                nc.vector.tensor_add(out=me[:, 1:2], in0=mv[:, 1:2], in1=m2)
                gs_ps = psB.tile([GPC, 2], F32, tag="pt")
                nc.tensor.matmul(out=gs_ps, lhsT=gmask, rhs=me, start=True, stop=True)
                gs = stp.tile([GPC, 2], F32, tag="gs")
                nc.vector.tensor_copy(out=gs, in_=gs_ps)
                var = stp.tile([GPC, 1], F32, tag="var")
                nc.vector.tensor_mul(out=var, in0=gs[:, 0:1], in1=gs[:, 0:1])
                var2 = stp.tile([GPC, 1], F32, tag="var2")
                nc.vector.tensor_sub(out=var2, in0=gs[:, 1:2], in1=var)
                sd = stp.tile([GPC, 1], F32, tag="sd")
                nc.scalar.activation(out=sd, in_=var2, func=AF.Sqrt, bias=eps_t)
                ab = stp.tile([GPC, 2], F32, tag="ab")
                nc.vector.reciprocal(out=ab[:, 0:1], in_=sd)
                nc.vector.scalar_tensor_tensor(
                    out=ab[:, 1:2], in0=gs[:, 0:1], scalar=-1.0, in1=ab[:, 0:1],
                    op0=A.mult, op1=A.mult,
                )
                abc_ps = psB.tile([P, 2], F32, tag="pt")
                nc.tensor.matmul(out=abc_ps, lhsT=bmask, rhs=ab, start=True, stop=True)
                abc = stp.tile([P, 2], F32, tag="abc")
                nc.vector.tensor_copy(out=abc, in_=abc_ps)
                nc.vector.tensor_scalar(
                    out=h_sb[:, k, :], in0=xc,
                    scalar1=abc[:, 0:1], scalar2=abc[:, 1:2],
                    op0=A.mult, op1=A.add,
                )

            # ---- q, k projections (c-major layout) ----
            q_sb = qp.tile([P, CCH, L1], BF16, tag="q")
            k_sb = kp.tile([P, CCH, L1], BF16, tag="k")
            for wname, dst, bname in (("wq", q_sb, "bq"), ("wk", k_sb, "bk")):
                for m in range(CCH):
                    ps = psA.tile([P, L1], F32, tag="mm")
                    for kk in range(CCH):
                        for nb in range(2):
                            nc.tensor.matmul(
                                out=ps[:, nb * 512:(nb + 1) * 512],
                                lhsT=w_sb[wname][:, kk, m * P:(m + 1) * P],
                                rhs=h_sb[:, kk, nb * 512:(nb + 1) * 512],
                                start=(kk == 0), stop=(kk == CCH - 1),
                            )
                    nc.any.tensor_scalar_add(
                        out=dst[:, m, :], in0=ps,
                        scalar1=b_sb[bname][:, m:m + 1],
                    )

            # ---- v^T (positions on partitions) ----
            vT_sb = vp.tile([P, LCH1, C], BF16, tag="v")
            for m in range(LCH1):
                ps = psB.tile([P, C], F32, tag="vT")
                for kk in range(CCH):
                    nc.tensor.matmul(
                        out=ps,
                        lhsT=h_sb[:, kk, m * P:(m + 1) * P],
                        rhs=w_sb["wv"][:, kk, :],
                        start=(kk == 0), stop=(kk == CCH - 1),
                    )
                nc.any.tensor_copy(out=vT_sb[:, m, :], in_=ps)

            # ---- S = q^T k, softmax, P^T (normalized via diag trick) ----
            pt_sb = ptp.tile([P, LCH1, L1], BF16, tag="ptv")
            for m in range(LCH1):
                ps_s = psA.tile([P, L1], F32, tag="mm")
                for kk in range(CCH):
                    for nb in range(2):
                        nc.tensor.matmul(
                            out=ps_s[:, nb * 512:(nb + 1) * 512],
                            lhsT=q_sb[:, kk, m * P:(m + 1) * P],
                            rhs=k_sb[:, kk, nb * 512:(nb + 1) * 512],
                            start=(kk == 0), stop=(kk == CCH - 1),
                        )
                p_sb = pp.tile([P, L1], BF16, tag="pv")
                rs = stp.tile([P, 1], F32, tag="rs")
                nc.scalar.activation(
                    out=p_sb, in_=ps_s, func=AF.Exp, accum_out=rs
                )
                rc = stp.tile([P, 1], F32, tag="rc")
                nc.vector.reciprocal(out=rc, in_=rs)
                dg = stp.tile([P, P], BF16, tag="dg")
                nc.vector.tensor_scalar_mul(out=dg, in0=ident, scalar1=rc)
                for j in range(LCH1):
                    ps_t = psB.tile([P, P], F32, tag="pt")
                    nc.tensor.matmul(
                        out=ps_t, lhsT=p_sb[:, j * P:(j + 1) * P], rhs=dg,
                        start=True, stop=True,
                    )
                    nc.any.tensor_copy(out=pt_sb[:, j, m * P:(m + 1) * P], in_=ps_t)

            # ---- O = v P^T (c-major out), +bv via rowsum=1 ----
            o_sb = hp.tile([P, CCH, L1], BF16, tag="h")
            for m in range(CCH):
                ps_o = psA.tile([P, L1], F32, tag="mm")
                for j in range(LCH1):
                    for nb in range(2):
                        nc.tensor.matmul(
                            out=ps_o[:, nb * 512:(nb + 1) * 512],
                            lhsT=vT_sb[:, j, m * P:(m + 1) * P],
                            rhs=pt_sb[:, j, nb * 512:(nb + 1) * 512],
                            start=(j == 0), stop=(j == LCH1 - 1),
                        )
                nc.any.tensor_scalar_add(
                    out=o_sb[:, m, :], in0=ps_o,
                    scalar1=b_sb["bv"][:, m:m + 1],
                )

            # ---- r = Wo O + bo + x -> ys ----
            for m in range(CCH):
                ps_r = psA.tile([P, L1], F32, tag="mm")
                for kk in range(CCH):
                    for nb in range(2):
                        nc.tensor.matmul(
                            out=ps_r[:, nb * 512:(nb + 1) * 512],
                            lhsT=w_sb["wo"][:, kk, m * P:(m + 1) * P],
                            rhs=o_sb[:, kk, nb * 512:(nb + 1) * 512],
                            start=(kk == 0), stop=(kk == CCH - 1),
                        )
                y_sb = yp.tile([P, L1], F32, tag="y")
                nc.vector.scalar_tensor_tensor(
                    out=y_sb, in0=ps_r, scalar=b_sb["bo"][:, m:m + 1],
                    in1=x_sb[:, m, :], op0=A.add, op1=A.add,
                )
                nc.sync.dma_start(out=ys[i, m * P:(m + 1) * P, :], in_=y_sb)
    return nc


# ---------------------------------------------------------------- phase 2
def build_temporal(reps=1):
    nc = bass.Bass()
    xt = nc.dram_tensor("xt", [C, NS2 * NT2], F32, kind="ExternalInput")
    yt = nc.dram_tensor("yt", [C, NS2 * NT2], F32, kind="ExternalOutput")
    wd = {
        n: nc.dram_tensor(n, [C, C], BF16, kind="ExternalInput")
        for n in ("wq", "wk", "wv", "wo")
    }
    bd = {
        n: nc.dram_tensor(n, [P, CCH], F32, kind="ExternalInput")
        for n in ("bq", "bk", "bv", "bo")
    }
    gmask_d = nc.dram_tensor("gmask", [P, GPC], F32, kind="ExternalInput")
    bmask_d = nc.dram_tensor("bmask", [GPC, P], F32, kind="ExternalInput")
    ident_d = nc.dram_tensor("ident", [P, P], BF16, kind="ExternalInput")
    blkmask_d = nc.dram_tensor("blkmask", [P, P], F32, kind="ExternalInput")
    A = _op()
    NN = HALF  # samples per half

    with tile.TileContext(nc) as tc, ExitStack() as ctx:
        const = ctx.enter_context(tc.tile_pool(name="const", bufs=1))
        stp = ctx.enter_context(tc.tile_pool(name="stats", bufs=3))
        xp = ctx.enter_context(tc.tile_pool(name="x", bufs=2))
        sqp = ctx.enter_context(tc.tile_pool(name="sq", bufs=2))
        tmpp = ctx.enter_context(tc.tile_pool(name="tmp", bufs=2))
        hp = ctx.enter_context(tc.tile_pool(name="h", bufs=1))
        qp = ctx.enter_context(tc.tile_pool(name="q", bufs=1))
        kp = ctx.enter_context(tc.tile_pool(name="k", bufs=1))
        vp = ctx.enter_context(tc.tile_pool(name="v", bufs=1))
        pp = ctx.enter_context(tc.tile_pool(name="pm", bufs=3))
        yp = ctx.enter_context(tc.tile_pool(name="y", bufs=3))
        psA = ctx.enter_context(tc.tile_pool(name="psA", bufs=2, space="PSUM"))
        psB = ctx.enter_context(tc.tile_pool(name="psB", bufs=2, space="PSUM"))

        w_sb = {}
        for n in wd:
            t = const.tile([P, CCH, C], BF16, tag=n)
            nc.sync.dma_start(out=t, in_=wd[n].rearrange("(k p) o -> p k o", p=P))
            w_sb[n] = t
        b_sb = {}
        for n in bd:
            t = const.tile([P, CCH], F32, tag=n)
            nc.sync.dma_start(out=t, in_=bd[n][:, :])
            b_sb[n] = t
        gmask = const.tile([P, GPC], F32, tag="gmask")
        nc.sync.dma_start(out=gmask, in_=gmask_d[:, :])
        bmask = const.tile([GPC, P], F32, tag="bmask")
        nc.sync.dma_start(out=bmask, in_=bmask_d[:, :])
        ident = const.tile([P, P], BF16, tag="ident")
        nc.sync.dma_start(out=ident, in_=ident_d[:, :])
        blkmask = const.tile([P, P], F32, tag="blkmask")
        nc.sync.dma_start(out=blkmask, in_=blkmask_d[:, :])
        eps_t = const.tile([GPC, 1], F32, tag="eps")
        nc.vector.memset(eps_t, EPS)

        xr = xt.rearrange("(k p) f -> p k f", p=P)
        yr = yt.rearrange("(k p) f -> p k f", p=P)

        for ih_rep in range(reps * 2):
            ih = ih_rep % 2
            f0 = ih * F2
            x_sb = xp.tile([P, CCH, F2], F32)
            nc.sync.dma_start(out=x_sb, in_=xr[:, :, f0:f0 + F2])

            # ---- GroupNorm over (16c x 16t) per sample ----
            h_sb = hp.tile([P, CCH, F2], BF16, tag="h")
            for k in range(CCH):
                xc = x_sb[:, k, :]
                xc3 = x_sb[:, k, :].rearrange("p (n t) -> p n t", t=NT2)
                sq = sqp.tile([P, F2], BF16, tag="sq")
                nc.scalar.activation(out=sq, in_=xc, func=AF.Square)
                me = stp.tile([P, 2, NN], F32, tag="me2")
                nc.vector.reduce_sum(out=me[:, 0, :], in_=xc3, axis=AX)
                nc.vector.reduce_sum(
                    out=me[:, 1, :],
                    in_=sq.rearrange("p (n t) -> p n t", t=NT2), axis=AX,
                )
                gs_ps = psB.tile([GPC, 2, NN], F32, tag="pt")
                nc.tensor.matmul(
                    out=gs_ps.rearrange("g a n -> g (a n)"),
                    lhsT=gmask, rhs=me.rearrange("p a n -> p (a n)"),
                    start=True, stop=True,
                )
                gs = stp.tile([GPC, 2, NN], F32, tag="gs2")
                nc.vector.tensor_copy(out=gs, in_=gs_ps)
                var = stp.tile([GPC, NN], F32, tag="var2a")
                nc.vector.tensor_mul(out=var, in0=gs[:, 0, :], in1=gs[:, 0, :])
                var2 = stp.tile([GPC, NN], F32, tag="var2b")
                nc.vector.tensor_sub(out=var2, in0=gs[:, 1, :], in1=var)
                sd = stp.tile([GPC, NN], F32, tag="sd2")
                nc.scalar.activation(out=sd, in_=var2, func=AF.Sqrt, bias=eps_t)
                ab = stp.tile([GPC, 2, NN], F32, tag="ab2")
                nc.vector.reciprocal(out=ab[:, 0, :], in_=sd)
                nc.vector.scalar_tensor_tensor(
                    out=ab[:, 1, :], in0=gs[:, 0, :], scalar=-1.0, in1=ab[:, 0, :],
                    op0=A.mult, op1=A.mult,
                )
                abc_ps = psB.tile([P, 2, NN], F32, tag="pt")
                nc.tensor.matmul(
                    out=abc_ps.rearrange("p a n -> p (a n)"),
                    lhsT=bmask, rhs=ab.rearrange("g a n -> g (a n)"),
                    start=True, stop=True,
                )
                abc = stp.tile([P, 2, NN], F32, tag="abc2")
                nc.vector.tensor_copy(out=abc, in_=abc_ps)
                tmp = tmpp.tile([P, F2], BF16, tag="tmp")
                nc.vector.tensor_tensor(
                    out=tmp.rearrange("p (n t) -> p n t", t=NT2),
                    in0=xc3, in1=_bcast_inner(abc[:, 0, :], NT2), op=A.mult,
                )
                nc.vector.tensor_tensor(
                    out=h_sb[:, k, :].rearrange("p (n t) -> p n t", t=NT2),
                    in0=tmp.rearrange("p (n t) -> p n t", t=NT2),
                    in1=_bcast_inner(abc[:, 1, :], NT2), op=A.add,
                )

            # ---- q, k projections ----
            q_sb = qp.tile([P, CCH, F2], BF16, tag="q")
            k_sb = kp.tile([P, CCH, F2], BF16, tag="k")
            for wname, dst, bname in (("wq", q_sb, "bq"), ("wk", k_sb, "bk")):
                for m in range(CCH):
                    for nb in range(NB2):
                        ps = psA.tile([P, 512], F32, tag="mm")
                        for kk in range(CCH):
                            nc.tensor.matmul(
                                out=ps,
                                lhsT=w_sb[wname][:, kk, m * P:(m + 1) * P],
                                rhs=h_sb[:, kk, nb * 512:(nb + 1) * 512],
                                start=(kk == 0), stop=(kk == CCH - 1),
                            )
                        nc.any.tensor_scalar_add(
                            out=dst[:, m, nb * 512:(nb + 1) * 512], in0=ps,
                            scalar1=b_sb[bname][:, m:m + 1],
                        )

            # ---- v^T ----
            vT_sb = vp.tile([P, NGRP, C], BF16, tag="v")
            for m in range(NGRP):
                ps = psB.tile([P, C], F32, tag="vT")
                for kk in range(CCH):
                    nc.tensor.matmul(
                        out=ps,
                        lhsT=h_sb[:, kk, m * P:(m + 1) * P],
                        rhs=w_sb["wv"][:, kk, :],
                        start=(kk == 0), stop=(kk == CCH - 1),
                    )
                nc.any.tensor_copy(out=vT_sb[:, m, :], in_=ps)

            # ---- attention per 8-sample group ----
            o_sb = hp.tile([P, CCH, F2], BF16, tag="h")
            for g in range(NGRP):
                c0 = g * P
                ps_s = psB.tile([P, P], F32, tag="so")
                for kk in range(CCH):
                    nc.tensor.matmul(
                        out=ps_s,
                        lhsT=q_sb[:, kk, c0:c0 + P],
                        rhs=k_sb[:, kk, c0:c0 + P],
                        start=(kk == 0), stop=(kk == CCH - 1),
                    )
                nc.vector.tensor_add(out=ps_s, in0=ps_s, in1=blkmask)
                p_sb = pp.tile([P, P], BF16, tag="pv")
                rs = stp.tile([P, 1], F32, tag="rs")
                nc.scalar.activation(
                    out=p_sb, in_=ps_s, func=AF.Exp, accum_out=rs
                )
                rc = stp.tile([P, 1], F32, tag="rc")
                nc.vector.reciprocal(out=rc, in_=rs)
                dg = stp.tile([P, P], BF16, tag="dg")
                nc.vector.tensor_scalar_mul(out=dg, in0=ident, scalar1=rc)
                ps_t = psB.tile([P, P], F32, tag="pt")
                nc.tensor.matmul(out=ps_t, lhsT=p_sb, rhs=dg, start=True, stop=True)
                pt_sb = pp.tile([P, P], BF16, tag="ptv")
                nc.vector.tensor_copy(out=pt_sb, in_=ps_t)
                for m in range(CCH):
                    ps_o = psB.tile([P, P], F32, tag="so")
                    nc.tensor.matmul(
                        out=ps_o, lhsT=vT_sb[:, g, m * P:(m + 1) * P], rhs=pt_sb,
                        start=True, stop=True,
                    )
                    nc.any.tensor_scalar_add(
                        out=o_sb[:, m, c0:c0 + P], in0=ps_o,
                        scalar1=b_sb["bv"][:, m:m + 1],
                    )

            # ---- r = Wo O + bo + x -> yt ----
            for m in range(CCH):
                for nb in range(NB2):
                    ps_r = psA.tile([P, 512], F32, tag="mm")
                    for kk in range(CCH):
                        nc.tensor.matmul(
                            out=ps_r,
                            lhsT=w_sb["wo"][:, kk, m * P:(m + 1) * P],
                            rhs=o_sb[:, kk, nb * 512:(nb + 1) * 512],
                            start=(kk == 0), stop=(kk == CCH - 1),
                        )
                    y_sb = yp.tile([P, 512], F32, tag="y")
                    nc.vector.scalar_tensor_tensor(
                        out=y_sb, in0=ps_r, scalar=b_sb["bo"][:, m:m + 1],
                        in1=x_sb[:, m, nb * 512:(nb + 1) * 512],
                        op0=A.add, op1=A.add,
                    )
                    nc.sync.dma_start(
                        out=yr[:, m, f0 + nb * 512:f0 + (nb + 1) * 512], in_=y_sb
                    )
    return nc


# ---------------------------------------------------------------- host side
def _fold_weights(w, b, gamma, beta, scale=1.0):
    """GN affine folded into conv: W @ (hn*gamma+beta) + b
    = (W*gamma) @ hn + (W@beta + b).  Returns (lhsT bf16 (c,o), bias (128,4))."""
    w = np.asarray(w, np.float32)
    b = np.asarray(b, np.float32)
    gamma = np.asarray(gamma, np.float32)
    beta = np.asarray(beta, np.float32)
    w_eff = w * gamma[None, :] * scale
    b_eff = (b + w @ beta) * scale
    wT = np.ascontiguousarray(w_eff.T).astype(ml_dtypes.bfloat16)
    bb = np.ascontiguousarray(b_eff.reshape(CCH, P).T)
    return wT, bb


def _consts():
    gmask1 = np.zeros((P, GPC), np.float32)
    for p in range(P):
        gmask1[p, p // GS] = 1.0 / (GS * 1)  # spatial: /16 (channel avg of means)
    gmask2 = np.zeros((P, GPC), np.float32)
    for p in range(P):
        gmask2[p, p // GS] = 1.0 / (GS * NT2)  # temporal: /256 (full group sum)
    bmask = np.zeros((GPC, P), np.float32)
    for p in range(P):
        bmask[p // GS, p] = 1.0
    ident = np.eye(P).astype(ml_dtypes.bfloat16)
    blk = np.full((P, P), -1e9, np.float32)
    for n in range(P // NT2):
        blk[n * NT2:(n + 1) * NT2, n * NT2:(n + 1) * NT2] = 0.0
    return gmask1, gmask2, bmask, ident, blk


_CACHE = {}


def kernel(**inputs):
    x = np.asarray(inputs["x"], np.float32)
    gmask1, gmask2, bmask, ident, blk = _consts()
    scale = float(C) ** -0.5

    wq1, bq1 = _fold_weights(inputs["wq_s"], inputs["bq_s"],
                             inputs["gamma_s"], inputs["beta_s"], scale)
    wk1, bk1 = _fold_weights(inputs["wk_s"], inputs["bk_s"],
                             inputs["gamma_s"], inputs["beta_s"])
    wv1, bv1 = _fold_weights(inputs["wv_s"], inputs["bv_s"],
                             inputs["gamma_s"], inputs["beta_s"])
    wo1, bo1 = _fold_weights(inputs["wo_s"], inputs["bo_s"],
                             np.ones(C, np.float32), np.zeros(C, np.float32))
    wq2, bq2 = _fold_weights(inputs["wq_t"], inputs["bq_t"],
                             inputs["gamma_t"], inputs["beta_t"], scale)
    wk2, bk2 = _fold_weights(inputs["wk_t"], inputs["bk_t"],
                             inputs["gamma_t"], inputs["beta_t"])
    wv2, bv2 = _fold_weights(inputs["wv_t"], inputs["bv_t"],
                             inputs["gamma_t"], inputs["beta_t"])
    wo2, bo2 = _fold_weights(inputs["wo_t"], inputs["bo_t"],
                             np.ones(C, np.float32), np.zeros(C, np.float32))

    if "nc1" not in _CACHE:
        _CACHE["nc1"] = _split_waits(build_spatial())
        _CACHE["nc2"] = _split_waits(build_temporal())
    nc1, nc2 = _CACHE["nc1"], _CACHE["nc2"]

    # ---- phase 1: spatial over (b t) ----
    xs = np.ascontiguousarray(
        x.transpose(0, 2, 1, 3, 4).reshape(B * T, C, L1)
    )
    common1 = dict(wq=wq1, wk=wk1, wv=wv1, wo=wo1,
                   bq=bq1, bk=bk1, bv=bv1, bo=bo1,
                   gmask=gmask1, bmask=bmask, ident=ident)
    in_maps1 = [
        dict(xs=np.ascontiguousarray(xs[i * NS1:(i + 1) * NS1]), **common1)
        for i in range(N_CORES)
    ]
    _CACHE["in_maps1"] = in_maps1
    r1 = run_bass_kernel_spmd(nc1, in_maps1, core_ids=list(range(N_CORES)),
                              **_CACHE.get("run_kwargs", {}))
    ys = np.concatenate([r1.results[i]["ys"] for i in range(N_CORES)], axis=0)
    _CACHE["last_r1"] = r1

    # ---- phase 2: temporal over (b h w) ----
    x2 = ys.reshape(B, T, C, H, W).transpose(0, 3, 4, 2, 1)  # (b,h,w,c,t)
    x2 = x2.reshape(B * H * W, C, NT2)
    common2 = dict(wq=wq2, wk=wk2, wv=wv2, wo=wo2,
                   bq=bq2, bk=bk2, bv=bv2, bo=bo2,
                   gmask=gmask2, bmask=bmask, ident=ident, blkmask=blk)
    in_maps2 = []
    for i in range(N_CORES):
        shard = x2[i * NS2:(i + 1) * NS2]          # (256, 512, 16)
        xt = np.ascontiguousarray(shard.transpose(1, 0, 2)).reshape(C, NS2 * NT2)
        in_maps2.append(dict(xt=xt, **common2))
    _CACHE["in_maps2"] = in_maps2
    r2 = run_bass_kernel_spmd(nc2, in_maps2, core_ids=list(range(N_CORES)),
                              **_CACHE.get("run_kwargs", {}))
    _CACHE["last_r2"] = r2

    out = np.empty((B * H * W, C, NT2), np.float32)
    for i in range(N_CORES):
        yt = r2.results[i]["yt"].reshape(C, NS2, NT2)
        out[i * NS2:(i + 1) * NS2] = yt.transpose(1, 0, 2)
    out = out.reshape(B, H, W, C, NT2).transpose(0, 3, 4, 1, 2)
    return np.ascontiguousarray(out)



# revision 12
# speedup vs baseline: 1.3229x; 1.3229x over previous
"""AttnBlockST Trainium2 kernel (fp8 DoubleRow version).

Two SPMD phases on 8 NeuronCores:
  phase 1 (spatial): data-parallel over b*t (32 samples -> 4/core),
    attention over hw=1024 within each (bt, c, hw) sample.
  phase 2 (temporal): data-parallel over b*h*w (2048 -> 256/core),
    attention over t=16, 8 samples packed per 128-partition group with a
    block-diagonal logit mask.

All large matmuls run in fp8e4 with DoubleRow perf mode (256-wide
contraction per pass, 0.5 cyc/row).  Weights are scaled by S_W=32 into
fp8 range host-side; activations q/k/v carry the S_W factor and the
logit scale c^-0.5/S_W^2 is applied inside the softmax exp.  GroupNorm
statistics, softmax and all accumulation stay fp32.  rstd is computed as
exp(-0.5*ln(var+eps)) so the scalar engine only ever needs the
ln/exp/identity activation table (no table thrashing).
"""

import numpy as np
import ml_dtypes
from contextlib import ExitStack

import concourse.bass as bass
import concourse.mybir as mybir
import concourse.tile as tile
from concourse.bass_utils import run_bass_kernel_spmd

# ---- walrus workaround: split multi-wait final drain ----
from concourse.vector_clock import ScopedClock
from concourse.tile import TileContext


def _patched_drain_and_barrier(self, tick_clock, wait_clock):
    nc = self.nc
    drain_inst = nc.sync.drain()
    wait_clock.add_sem_waits(
        drain_inst.ins, ScopedClock({None: tick_clock.global_clock})
    )
    si = drain_inst.ins.sync_info
    if si is not None and len(si.on_wait) > 1:
        waits = list(si.on_wait)
        drain_inst.ins.sync_info = mybir.SyncInfo(
            on_wait=waits[:1], on_update=list(si.on_update)
        )
        for w in waits[1:]:
            n = nc.sync.nop(nofuse=True, hint="drain_wait_split")
            n.ins.sync_info = mybir.SyncInfo(on_wait=[w], on_update=[])
    nc.all_engine_barrier()
    assert self.sems is not None
    popped = nc._tile_sem_poison_stack.pop()
    assert popped is self._sem_poison
    nc.clear_and_free_semaphores(list(self.sems.allocated().values()))
    nc.all_engine_barrier()


TileContext._drain_and_barrier = _patched_drain_and_barrier

# ---- problem constants (hardcoded per spec) ----
B, C, T, H, W = 2, 512, 16, 32, 32
GROUPS = 32
EPS = 1e-6
N_CORES = 8
P = 128
CCH = C // P          # 4 channel chunks
GPC = GROUPS // CCH   # 8 groups per 128-channel chunk
GS = C // GROUPS      # 16 channels per group

L1 = H * W            # 1024 spatial positions
NS1 = (B * T) // N_CORES   # 4 samples per core, phase 1
LCH1 = L1 // P        # 8 position chunks

NT2 = 16              # temporal length
NS2 = (B * H * W) // N_CORES  # 256 samples per core, phase 2
HALF = NS2 // 2       # process in halves of 128 samples
F2 = HALF * NT2       # 2048 free columns per half
NGRP = F2 // P        # 16 groups of 8 samples per half
GB = 8                # groups per attention sub-batch

S_W = 32.0            # fp8 weight scale
DG_S = 256.0          # diag (1/rowsum) scale into fp8 range
SC_EXP = float(C) ** -0.5 / (S_W * S_W)   # logit scale applied in exp

F32 = mybir.dt.float32
BF16 = mybir.dt.bfloat16
F8 = mybir.dt.float8e4
AX = mybir.AxisListType.X
AF = mybir.ActivationFunctionType
DR = mybir.MatmulPerfMode.DoubleRow


def _op():
    from concourse.alu_op_type import AluOpType
    return AluOpType


def _bcast_inner(ap, n):
    """View (P, F) access pattern as (P, F, n) with stride-0 inner dim."""
    return bass.AP(tensor=ap.tensor, offset=ap.offset, ap=list(ap.ap) + [[0, n]])


def _bcast_outer(ap, n):
    """View (P, F) access pattern as (P, n, F) with stride-0 middle dim."""
    a = list(ap.ap)
    return bass.AP(tensor=ap.tensor, offset=ap.offset,
                   ap=[a[0], [0, n]] + a[1:])


def _split_waits(nc, limit=1):
    """This walrus build rejects >1 sem wait on every ISA template tested
    (LDWEIGHTS, CTRL, ACT, DVE TensorScalar); hoist extra waits onto
    same-engine NoOps placed just before."""
    ctr = [0]
    for f in nc.m.functions:
        for b in f.blocks:
            new = []
            for ins in b.instructions:
                si = getattr(ins, "sync_info", None)
                waits = list(si.on_wait) if si is not None and si.on_wait else []
                lim = limit
                if len(waits) > lim:
                    for w in waits[lim:]:
                        ctr[0] += 1
                        new.append(mybir.InstNoOp(
                            name=f"wsplit-{ctr[0]}",
                            sync_info=mybir.SyncInfo(on_wait=[w], on_update=[]),
                            bass_nofuse=True,
                            engine=ins.engine,
                        ))
                    ins.sync_info = mybir.SyncInfo(
                        on_wait=waits[:lim], on_update=list(si.on_update)
                    )
                new.append(ins)
            b.instructions = new
    return nc


# ---------------------------------------------------------------- phase 1
def build_spatial(reps=1):
    nc = bass.Bass()
    xs = nc.dram_tensor("xs", [NS1, C, L1], BF16, kind="ExternalInput")
    ys = nc.dram_tensor("ys", [NS1, C, L1], F32, kind="ExternalOutput")
    wd = {
        n: nc.dram_tensor(n, [C, C], F8, kind="ExternalInput")
        for n in ("wq", "wk", "wv", "wo")
    }
    bd = {
        n: nc.dram_tensor(n, [P, CCH], F32, kind="ExternalInput")
        for n in ("bq", "bv", "bo")
    }
    gmask_d = nc.dram_tensor("gmask", [P, GPC], F32, kind="ExternalInput")
    bmask_d = nc.dram_tensor("bmask", [GPC, P], F32, kind="ExternalInput")
    ident_d = nc.dram_tensor("ident", [P, P], BF16, kind="ExternalInput")
    A = _op()

    with tile.TileContext(nc) as tc, ExitStack() as ctx:
        const = ctx.enter_context(tc.tile_pool(name="const", bufs=1))
        stp = ctx.enter_context(tc.tile_pool(name="stats", bufs=3))
        xp = ctx.enter_context(tc.tile_pool(name="x", bufs=2))
        hp = ctx.enter_context(tc.tile_pool(name="h", bufs=2))
        qp = ctx.enter_context(tc.tile_pool(name="q", bufs=2))
        kp = ctx.enter_context(tc.tile_pool(name="k", bufs=2))
        vp = ctx.enter_context(tc.tile_pool(name="v", bufs=2))
        pp = ctx.enter_context(tc.tile_pool(name="pm", bufs=3))
        ptp = ctx.enter_context(tc.tile_pool(name="pt", bufs=2))
        yp = ctx.enter_context(tc.tile_pool(name="y", bufs=3))
        psA = ctx.enter_context(tc.tile_pool(name="psA", bufs=2, space="PSUM"))
        psB = ctx.enter_context(tc.tile_pool(name="psB", bufs=2, space="PSUM"))

        w_sb = {}
        for n in wd:
            t = const.tile([P, CCH, C], F8, tag=n)
            nc.sync.dma_start(out=t, in_=wd[n].rearrange("(k p) o -> p k o", p=P))
            w_sb[n] = t
        b_sb = {}
        for n in bd:
            t = const.tile([P, CCH], F32, tag=n)
            nc.sync.dma_start(out=t, in_=bd[n][:, :])
            b_sb[n] = t
        gmask = const.tile([P, GPC], F32, tag="gmask")
        nc.sync.dma_start(out=gmask, in_=gmask_d[:, :])
        bmask = const.tile([GPC, P], F32, tag="bmask")
        nc.sync.dma_start(out=bmask, in_=bmask_d[:, :])
        ident = const.tile([P, P], BF16, tag="ident")
        nc.sync.dma_start(out=ident, in_=ident_d[:, :])
        eps_t = const.tile([GPC, 1], F32, tag="eps")
        nc.vector.memset(eps_t, EPS)

        for i_rep in range(reps * NS1):
            i = i_rep % NS1
            x_sb = xp.tile([P, CCH, L1], BF16)
            nc.sync.dma_start(out=x_sb, in_=xs[i].rearrange("(k p) l -> p k l", p=P))

            # ---- GroupNorm -> h (fp8) ----
            h_sb = hp.tile([P, CCH, L1], F8, tag="h")
            for k in range(CCH):
                xc = x_sb[:, k, :]
                st = stp.tile([P, 2, 6], F32, tag="bnst")
                nc.vector.bn_stats(out=st[:, 0, :], in_=xc[:, 0:512])
                nc.vector.bn_stats(out=st[:, 1, :], in_=xc[:, 512:1024])
                mv = stp.tile([P, 2], F32, tag="mv")
                nc.vector.bn_aggr(out=mv, in_=st)
                me = stp.tile([P, 2], F32, tag="me")
                nc.vector.tensor_copy(out=me[:, 0:1], in_=mv[:, 0:1])
                m2 = stp.tile([P, 1], F32, tag="m2")
                nc.vector.tensor_mul(out=m2, in0=mv[:, 0:1], in1=mv[:, 0:1])
                nc.vector.tensor_add(out=me[:, 1:2], in0=mv[:, 1:2], in1=m2)
                gs_ps = psB.tile([GPC, 2], F32, tag="vT")
                nc.tensor.matmul(out=gs_ps, lhsT=gmask, rhs=me, start=True, stop=True)
                gs = stp.tile([GPC, 2], F32, tag="gs")
                nc.vector.tensor_copy(out=gs, in_=gs_ps)
                var = stp.tile([GPC, 1], F32, tag="var")
                nc.vector.tensor_mul(out=var, in0=gs[:, 0:1], in1=gs[:, 0:1])
                var2 = stp.tile([GPC, 1], F32, tag="var2")
                nc.vector.tensor_sub(out=var2, in0=gs[:, 1:2], in1=var)
                # rstd = exp(-0.5*ln(var+eps)) -- stays on the ln/exp table
                lnv = stp.tile([GPC, 1], F32, tag="lnv")
                nc.scalar.activation(out=lnv, in_=var2, func=AF.Ln, bias=eps_t)
                ab = stp.tile([GPC, 2], F32, tag="ab")
                nc.scalar.activation(out=ab[:, 0:1], in_=lnv, func=AF.Exp,
                                     scale=-0.5)
                nc.vector.scalar_tensor_tensor(
                    out=ab[:, 1:2], in0=gs[:, 0:1], scalar=-1.0, in1=ab[:, 0:1],
                    op0=A.mult, op1=A.mult,
                )
                abc_ps = psB.tile([P, 2], F32, tag="vT")
                nc.tensor.matmul(out=abc_ps, lhsT=bmask, rhs=ab, start=True, stop=True)
                abc = stp.tile([P, 2], F32, tag="abc")
                nc.vector.tensor_copy(out=abc, in_=abc_ps)
                nc.gpsimd.tensor_scalar(
                    out=h_sb[:, k, :], in0=xc,
                    scalar1=abc[:, 0:1], scalar2=abc[:, 1:2],
                    op0=A.mult, op1=A.add,
                )

            # ---- q, k projections (c-major, fp8 DoubleRow) ----
            q_sb = qp.tile([P, CCH, L1], F8, tag="q")
            k_sb = kp.tile([P, CCH, L1], F8, tag="k")
            for wname, dst in (("wq", q_sb), ("wk", k_sb)):
                for m in range(CCH):
                    ps = psA.tile([P, L1], F32, tag="mm")
                    for kk in (0, 2):
                        for nb in range(2):
                            nc.tensor.matmul(
                                out=ps[:, nb * 512:(nb + 1) * 512],
                                lhsT=w_sb[wname][:, kk:kk + 2, m * P:(m + 1) * P],
                                rhs=h_sb[:, kk:kk + 2, nb * 512:(nb + 1) * 512],
                                start=(kk == 0), stop=(kk == 2),
                                perf_mode=DR,
                            )
                    if wname == "wq":
                        nc.vector.tensor_scalar_add(
                            out=dst[:, m, :], in0=ps,
                            scalar1=b_sb["bq"][:, m:m + 1],
                        )
                    else:
                        # bk shifts every logit row by a constant -> cancels
                        # in softmax; skip the bias entirely.
                        nc.vector.tensor_copy(out=dst[:, m, :], in_=ps)

            # ---- v^T (positions on partitions, fp8 DoubleRow) ----
            vT_sb = vp.tile([P, LCH1, C], F8, tag="v")
            for m in range(LCH1):
                ps = psB.tile([P, C], F32, tag="vT")
                for kk in (0, 2):
                    nc.tensor.matmul(
                        out=ps,
                        lhsT=h_sb[:, kk:kk + 2, m * P:(m + 1) * P],
                        rhs=w_sb["wv"][:, kk:kk + 2, :],
                        start=(kk == 0), stop=(kk == 2),
                        perf_mode=DR,
                    )
                nc.scalar.activation(out=vT_sb[:, m, :], in_=ps, func=AF.Copy)

            # ---- S = q^T k (DoubleRow), softmax, P^T (diag trick) ----
            pt_sb = ptp.tile([P, LCH1, L1], F8, tag="ptv")
            for m in range(LCH1):
                ps_s = psA.tile([P, L1], F32, tag="mm")
                for kk in (0, 2):
                    for nb in range(2):
                        nc.tensor.matmul(
                            out=ps_s[:, nb * 512:(nb + 1) * 512],
                            lhsT=q_sb[:, kk:kk + 2, m * P:(m + 1) * P],
                            rhs=k_sb[:, kk:kk + 2, nb * 512:(nb + 1) * 512],
                            start=(kk == 0), stop=(kk == 2),
                            perf_mode=DR,
                        )
                p_sb = pp.tile([P, L1], BF16, tag="pv")
                rs = stp.tile([P, 1], F32, tag="rs")
                nc.scalar.activation(
                    out=p_sb, in_=ps_s, func=AF.Exp, scale=SC_EXP, accum_out=rs
                )
                rc = stp.tile([P, 1], F32, tag="rc")
                nc.vector.reciprocal(out=rc, in_=rs)
                dg = stp.tile([P, P], BF16, tag="dg")
                nc.gpsimd.tensor_scalar(
                    out=dg, in0=ident, scalar1=rc, scalar2=DG_S,
                    op0=A.mult, op1=A.mult,
                )
                for jh in range(2):
                    ps_t = psB.tile([P, 512], F32, tag="pt")
                    for j4 in range(4):
                        j = jh * 4 + j4
                        nc.tensor.matmul(
                            out=ps_t[:, j4 * P:(j4 + 1) * P],
                            lhsT=p_sb[:, j * P:(j + 1) * P], rhs=dg,
                            start=True, stop=True,
                        )
                    dst = pt_sb[:, jh * 4:(jh + 1) * 4, m * P:(m + 1) * P]
                    src = ps_t.rearrange("p (j q) -> p j q", q=P)
                    if (m + jh) % 2 == 0:
                        nc.vector.tensor_copy(out=dst, in_=src)
                    else:
                        nc.scalar.activation(out=dst, in_=src, func=AF.Copy)

            # ---- O = v P^T (DoubleRow over position pairs) ----
            o_sb = hp.tile([P, CCH, L1], F8, tag="h")
            for m in range(CCH):
                ps_o = psA.tile([P, L1], F32, tag="mm")
                for jj in (0, 2, 4, 6):
                    for nb in range(2):
                        nc.tensor.matmul(
                            out=ps_o[:, nb * 512:(nb + 1) * 512],
                            lhsT=vT_sb[:, jj:jj + 2, m * P:(m + 1) * P],
                            rhs=pt_sb[:, jj:jj + 2, nb * 512:(nb + 1) * 512],
                            start=(jj == 0), stop=(jj == 6),
                            perf_mode=DR,
                        )
                nc.vector.tensor_scalar(
                    out=o_sb[:, m, :], in0=ps_o,
                    scalar1=1.0 / (S_W * DG_S), scalar2=b_sb["bv"][:, m:m + 1],
                    op0=A.mult, op1=A.add,
                )

            # ---- r = Wo O + bo + x -> ys ----
            for m in range(CCH):
                ps_r = psA.tile([P, L1], F32, tag="mm")
                for kk in (0, 2):
                    for nb in range(2):
                        nc.tensor.matmul(
                            out=ps_r[:, nb * 512:(nb + 1) * 512],
                            lhsT=w_sb["wo"][:, kk:kk + 2, m * P:(m + 1) * P],
                            rhs=o_sb[:, kk:kk + 2, nb * 512:(nb + 1) * 512],
                            start=(kk == 0), stop=(kk == 2),
                            perf_mode=DR,
                        )
                tmp = stp.tile([P, L1], BF16, tag="ytmp")
                nc.scalar.activation(out=tmp, in_=ps_r, func=AF.Identity,
                                     scale=1.0 / S_W, bias=b_sb["bo"][:, m:m + 1])
                y_sb = yp.tile([P, L1], F32, tag="y")
                nc.gpsimd.tensor_add(out=y_sb, in0=tmp, in1=x_sb[:, m, :])
                nc.sync.dma_start(out=ys[i, m * P:(m + 1) * P, :], in_=y_sb)
    return nc


# ---------------------------------------------------------------- phase 2
def build_temporal(reps=1):
    nc = bass.Bass()
    xt = nc.dram_tensor("xt", [C, NS2 * NT2], BF16, kind="ExternalInput")
    yt = nc.dram_tensor("yt", [C, NS2 * NT2], F32, kind="ExternalOutput")
    wd = {
        n: nc.dram_tensor(n, [C, C], F8, kind="ExternalInput")
        for n in ("wq", "wk", "wv", "wo")
    }
    bd = {
        n: nc.dram_tensor(n, [P, CCH], F32, kind="ExternalInput")
        for n in ("bq", "bv", "bo")
    }
    gmask_d = nc.dram_tensor("gmask", [P, GPC], BF16, kind="ExternalInput")
    bmask_d = nc.dram_tensor("bmask", [GPC, P], BF16, kind="ExternalInput")
    ident_d = nc.dram_tensor("ident", [P, P], BF16, kind="ExternalInput")
    blkmask_d = nc.dram_tensor("blkmask", [P, P], F32, kind="ExternalInput")
    A = _op()
    NN = HALF  # samples per half

    with tile.TileContext(nc) as tc, ExitStack() as ctx:
        const = ctx.enter_context(tc.tile_pool(name="const", bufs=1))
        stp = ctx.enter_context(tc.tile_pool(name="stats", bufs=2))
        xp = ctx.enter_context(tc.tile_pool(name="x", bufs=2))
        sqp = ctx.enter_context(tc.tile_pool(name="sq", bufs=2))
        tmpp = ctx.enter_context(tc.tile_pool(name="tmp", bufs=2))
        hp = ctx.enter_context(tc.tile_pool(name="h", bufs=2))
        qp = ctx.enter_context(tc.tile_pool(name="q", bufs=2))
        kp = ctx.enter_context(tc.tile_pool(name="k", bufs=2))
        vp = ctx.enter_context(tc.tile_pool(name="v", bufs=2))
        pp = ctx.enter_context(tc.tile_pool(name="pm", bufs=2))
        yp = ctx.enter_context(tc.tile_pool(name="y", bufs=2))
        psA = ctx.enter_context(tc.tile_pool(name="psA", bufs=2, space="PSUM"))
        psB = ctx.enter_context(tc.tile_pool(name="psB", bufs=2, space="PSUM"))

        w_sb = {}
        for n in wd:
            t = const.tile([P, CCH, C], F8, tag=n)
            nc.sync.dma_start(out=t, in_=wd[n].rearrange("(k p) o -> p k o", p=P))
            w_sb[n] = t
        b_sb = {}
        for n in bd:
            t = const.tile([P, CCH], F32, tag=n)
            nc.sync.dma_start(out=t, in_=bd[n][:, :])
            b_sb[n] = t
        gmask = const.tile([P, GPC], BF16, tag="gmask")
        nc.sync.dma_start(out=gmask, in_=gmask_d[:, :])
        bmask = const.tile([GPC, P], BF16, tag="bmask")
        nc.sync.dma_start(out=bmask, in_=bmask_d[:, :])
        ident = const.tile([P, P], BF16, tag="ident")
        nc.sync.dma_start(out=ident, in_=ident_d[:, :])
        ident256 = const.tile([P, P], BF16, tag="ident256")
        nc.vector.tensor_scalar_mul(out=ident256, in0=ident, scalar1=DG_S)
        blkmask = const.tile([P, P], F32, tag="blkmask")
        nc.sync.dma_start(out=blkmask, in_=blkmask_d[:, :])
        eps_t = const.tile([GPC, 1], F32, tag="eps")
        nc.vector.memset(eps_t, EPS)

        xr = xt.rearrange("(k p) f -> p k f", p=P)
        yr = yt.rearrange("(k p) f -> p k f", p=P)

        for ih_rep in range(reps * 2):
            ih = ih_rep % 2
            f0 = ih * F2
            x_sb = xp.tile([P, CCH, F2], BF16)
            nc.sync.dma_start(out=x_sb, in_=xr[:, :, f0:f0 + F2])

            # ---- GroupNorm stats over (16c x 16t) per sample ----
            me = stp.tile([P, 2, CCH, NN], F32, tag="me2")
            for k in range(CCH):
                xc3 = x_sb[:, k, :].rearrange("p (n t) -> p n t", t=NT2)
                sq = sqp.tile([P, F2], BF16, tag="sq")
                nc.gpsimd.tensor_mul(out=sq, in0=x_sb[:, k, :], in1=x_sb[:, k, :])
                nc.vector.reduce_sum(out=me[:, 0, k, :], in_=xc3, axis=AX)
                nc.vector.reduce_sum(
                    out=me[:, 1, k, :],
                    in_=sq.rearrange("p (n t) -> p n t", t=NT2), axis=AX,
                )
            me_bf = stp.tile([P, 2, CCH, NN], BF16, tag="mebf")
            nc.vector.tensor_copy(out=me_bf, in_=me)
            gs_ps = psB.tile([GPC, 2, CCH, NN], F32, tag="sps")
            for hb in range(2):
                nc.tensor.matmul(
                    out=gs_ps.rearrange("g a k n -> g (a k n)")[:, hb * 512:(hb + 1) * 512],
                    lhsT=gmask,
                    rhs=me_bf.rearrange("p a k n -> p (a k n)")[:, hb * 512:(hb + 1) * 512],
                    start=True, stop=True,
                )
            gs = stp.tile([GPC, 2, CCH, NN], F32, tag="gs2")
            nc.vector.tensor_copy(out=gs, in_=gs_ps)
            var = stp.tile([GPC, CCH, NN], F32, tag="var2a")
            nc.vector.tensor_mul(
                out=var, in0=gs[:, 0, :, :], in1=gs[:, 0, :, :])
            var2 = stp.tile([GPC, CCH, NN], F32, tag="var2b")
            nc.vector.tensor_sub(out=var2, in0=gs[:, 1, :, :], in1=var)
            lnv = stp.tile([GPC, CCH, NN], F32, tag="lnv")
            nc.scalar.activation(
                out=lnv, in_=var2.rearrange("g k n -> g (k n)"),
                func=AF.Ln, bias=eps_t)
            ab = stp.tile([GPC, 2, CCH, NN], BF16, tag="ab2")
            nc.scalar.activation(
                out=ab[:, 0, :, :], in_=lnv, func=AF.Exp, scale=-0.5)
            nc.vector.scalar_tensor_tensor(
                out=ab[:, 1, :, :], in0=gs[:, 0, :, :], scalar=-1.0,
                in1=ab[:, 0, :, :], op0=A.mult, op1=A.mult,
            )
            abc_ps = psB.tile([P, 2, CCH, NN], F32, tag="sps")
            for hb in range(2):
                nc.tensor.matmul(
                    out=abc_ps.rearrange("p a k n -> p (a k n)")[:, hb * 512:(hb + 1) * 512],
                    lhsT=bmask,
                    rhs=ab.rearrange("g a k n -> g (a k n)")[:, hb * 512:(hb + 1) * 512],
                    start=True, stop=True,
                )
            abc = stp.tile([P, 2, CCH, NN], BF16, tag="abc2")
            nc.vector.tensor_copy(out=abc, in_=abc_ps)

            # ---- GN apply -> h (fp8) ----
            h_sb = hp.tile([P, CCH, F2], F8, tag="h")
            for k in range(CCH):
                xc3 = x_sb[:, k, :].rearrange("p (n t) -> p n t", t=NT2)
                tmp = tmpp.tile([P, F2], BF16, tag="tmp")
                nc.vector.tensor_tensor(
                    out=tmp.rearrange("p (n t) -> p n t", t=NT2),
                    in0=xc3, in1=_bcast_inner(abc[:, 0, k, :], NT2), op=A.mult,
                )
                nc.gpsimd.tensor_tensor(
                    out=h_sb[:, k, :].rearrange("p (n t) -> p n t", t=NT2),
                    in0=tmp.rearrange("p (n t) -> p n t", t=NT2),
                    in1=_bcast_inner(abc[:, 1, k, :], NT2), op=A.add,
                )

            # ---- q, k projections (fp8 DoubleRow) ----
            q_sb = qp.tile([P, CCH, F2], F8, tag="q")
            k_sb = kp.tile([P, CCH, F2], F8, tag="k")
            for wname, dst in (("wq", q_sb), ("wk", k_sb)):
                for m in range(CCH):
                    for nbb in range(2):
                        ps = psA.tile([P, 1024], F32, tag="mm")
                        for kk in (0, 2):
                            for nb in range(2):
                                nc.tensor.matmul(
                                    out=ps[:, nb * 512:(nb + 1) * 512],
                                    lhsT=w_sb[wname][:, kk:kk + 2, m * P:(m + 1) * P],
                                    rhs=h_sb[:, kk:kk + 2,
                                             nbb * 1024 + nb * 512:
                                             nbb * 1024 + (nb + 1) * 512],
                                    start=(kk == 0), stop=(kk == 2),
                                    perf_mode=DR,
                                )
                        sl = slice(nbb * 1024, (nbb + 1) * 1024)
                        if wname == "wq":
                            nc.vector.tensor_scalar_add(
                                out=dst[:, m, sl], in0=ps,
                                scalar1=b_sb["bq"][:, m:m + 1],
                            )
                        else:
                            nc.vector.tensor_copy(out=dst[:, m, sl], in_=ps)

            # ---- v^T (fp8 DoubleRow), 2 groups per psum tile ----
            vT_sb = vp.tile([P, NGRP, C], F8, tag="v")
            for mg in range(NGRP // 2):
                ps = psA.tile([P, 1024], F32, tag="mm")
                for sub in range(2):
                    m = mg * 2 + sub
                    for kk in (0, 2):
                        nc.tensor.matmul(
                            out=ps[:, sub * 512:(sub + 1) * 512],
                            lhsT=h_sb[:, kk:kk + 2, m * P:(m + 1) * P],
                            rhs=w_sb["wv"][:, kk:kk + 2, :],
                            start=(kk == 0), stop=(kk == 2),
                            perf_mode=DR,
                        )
                nc.scalar.activation(
                    out=vT_sb[:, mg * 2:mg * 2 + 2, :],
                    in_=ps.rearrange("p (s c) -> p s c", c=C), func=AF.Copy)

            # ---- attention, GB-group batches ----
            o_sb = hp.tile([P, CCH, F2], F8, tag="h")
            for bat in range(NGRP // GB):
                g0 = bat * GB
                ps_s = psB.tile([P, GB * P], F32, tag="sps")
                for g in range(GB):
                    c0 = (g0 + g) * P
                    for kk in (0, 2):
                        nc.tensor.matmul(
                            out=ps_s[:, g * P:(g + 1) * P],
                            lhsT=q_sb[:, kk:kk + 2, c0:c0 + P],
                            rhs=k_sb[:, kk:kk + 2, c0:c0 + P],
                            start=(kk == 0), stop=(kk == 2),
                            perf_mode=DR,
                        )
                nc.vector.tensor_tensor(
                    out=ps_s.rearrange("p (g q) -> p g q", q=P),
                    in0=ps_s.rearrange("p (g q) -> p g q", q=P),
                    in1=_bcast_outer(blkmask, GB), op=A.add,
                )
                p_sb = pp.tile([P, GB * P], BF16, tag="pv")
                nc.scalar.activation(out=p_sb, in_=ps_s, func=AF.Exp,
                                     scale=SC_EXP)
                rs = stp.tile([P, GB], F32, tag="rs")
                nc.vector.reduce_sum(
                    out=rs, in_=p_sb.rearrange("p (g q) -> p g q", q=P), axis=AX)
                rc = stp.tile([P, GB], F32, tag="rc")
                nc.vector.reciprocal(out=rc, in_=rs)
                dg = stp.tile([P, GB, P], BF16, tag="dg")
                nc.gpsimd.tensor_tensor(
                    out=dg, in0=_bcast_inner(rc, P),
                    in1=_bcast_outer(ident256, GB), op=A.mult,
                )
                ps_t = psB.tile([P, GB * P], F32, tag="sps")
                for g in range(GB):
                    nc.tensor.matmul(
                        out=ps_t[:, g * P:(g + 1) * P],
                        lhsT=p_sb[:, g * P:(g + 1) * P], rhs=dg[:, g, :],
                        start=True, stop=True,
                    )
                pt_sb = pp.tile([P, GB * P], F8, tag="ptv")
                nc.vector.tensor_copy(out=pt_sb, in_=ps_t)
                for m in range(CCH):
                    ps_o = psA.tile([P, GB * P], F32, tag="mm")
                    for g in range(GB):
                        nc.tensor.matmul(
                            out=ps_o[:, g * P:(g + 1) * P],
                            lhsT=vT_sb[:, g0 + g, m * P:(m + 1) * P],
                            rhs=pt_sb[:, g * P:(g + 1) * P],
                            start=True, stop=True,
                        )
                    nc.scalar.activation(
                        out=o_sb[:, m, g0 * P:(g0 + GB) * P], in_=ps_o,
                        func=AF.Identity, scale=1.0 / (S_W * DG_S),
                        bias=b_sb["bv"][:, m:m + 1])

                # ---- r = Wo O + bo + x for this batch's columns ----
                for m in range(CCH):
                    ps_r = psA.tile([P, GB * P], F32, tag="mm")
                    for kk in (0, 2):
                        for nb in range(2):
                            nc.tensor.matmul(
                                out=ps_r[:, nb * 512:(nb + 1) * 512],
                                lhsT=w_sb["wo"][:, kk:kk + 2, m * P:(m + 1) * P],
                                rhs=o_sb[:, kk:kk + 2,
                                         g0 * P + nb * 512:
                                         g0 * P + (nb + 1) * 512],
                                start=(kk == 0), stop=(kk == 2),
                                perf_mode=DR,
                            )
                    tmp = tmpp.tile([P, GB * P], BF16, tag="ytmp")
                    nc.scalar.activation(out=tmp, in_=ps_r, func=AF.Identity,
                                         scale=1.0 / S_W,
                                         bias=b_sb["bo"][:, m:m + 1])
                    y_sb = yp.tile([P, GB * P], F32, tag="y")
                    nc.gpsimd.tensor_add(
                        out=y_sb, in0=tmp, in1=x_sb[:, m, g0 * P:(g0 + GB) * P])
                    nc.sync.dma_start(
                        out=yr[:, m, f0 + g0 * P:f0 + (g0 + GB) * P], in_=y_sb)
    return nc


# ---------------------------------------------------------------- host side
F8NP = mybir.dt.np(mybir.dt.float8e4)


def _fold_weights(w, b, gamma, beta):
    """GN affine folded into conv: W @ (hn*gamma+beta) + b
    = (W*gamma) @ hn + (W@beta + b).  Weights scaled by S_W into fp8;
    returns (lhsT fp8 (c,o), bias f32 (128,4))."""
    w = np.asarray(w, np.float32)
    b = np.asarray(b, np.float32)
    gamma = np.asarray(gamma, np.float32)
    beta = np.asarray(beta, np.float32)
    w_eff = w * gamma[None, :] * S_W
    b_eff = b + w @ beta
    wT = np.clip(np.ascontiguousarray(w_eff.T), -240., 240.).astype(F8NP)
    bb = np.ascontiguousarray(b_eff.reshape(CCH, P).T)
    return wT, bb


def _consts():
    gmask1 = np.zeros((P, GPC), np.float32)
    for p in range(P):
        gmask1[p, p // GS] = 1.0 / (GS * 1)  # spatial: /16 (channel avg of means)
    gmask2 = np.zeros((P, GPC), np.float32)
    for p in range(P):
        gmask2[p, p // GS] = 1.0 / (GS * NT2)  # temporal: /256 (full group sum)
    bmask = np.zeros((GPC, P), np.float32)
    for p in range(P):
        bmask[p // GS, p] = 1.0
    ident = np.eye(P).astype(ml_dtypes.bfloat16)
    blk = np.full((P, P), -1e9, np.float32)
    for n in range(P // NT2):
        blk[n * NT2:(n + 1) * NT2, n * NT2:(n + 1) * NT2] = 0.0
    return gmask1, gmask2, bmask, ident, blk


_CACHE = {}


def kernel(**inputs):
    x = np.asarray(inputs["x"], np.float32)
    gmask1, gmask2, bmask, ident, blk = _consts()

    wq1, bq1 = _fold_weights(inputs["wq_s"], inputs["bq_s"],
                             inputs["gamma_s"], inputs["beta_s"])
    wk1, _ = _fold_weights(inputs["wk_s"], inputs["bk_s"],
                           inputs["gamma_s"], inputs["beta_s"])
    wv1, bv1 = _fold_weights(inputs["wv_s"], inputs["bv_s"],
                             inputs["gamma_s"], inputs["beta_s"])
    wo1, bo1 = _fold_weights(inputs["wo_s"], inputs["bo_s"],
                             np.ones(C, np.float32), np.zeros(C, np.float32))
    wq2, bq2 = _fold_weights(inputs["wq_t"], inputs["bq_t"],
                             inputs["gamma_t"], inputs["beta_t"])
    wk2, _ = _fold_weights(inputs["wk_t"], inputs["bk_t"],
                           inputs["gamma_t"], inputs["beta_t"])
    wv2, bv2 = _fold_weights(inputs["wv_t"], inputs["bv_t"],
                             inputs["gamma_t"], inputs["beta_t"])
    wo2, bo2 = _fold_weights(inputs["wo_t"], inputs["bo_t"],
                             np.ones(C, np.float32), np.zeros(C, np.float32))
    # q bias is added to the S_W-scaled psum result
    bq1 = bq1 * S_W
    bq2 = bq2 * S_W

    if "nc1" not in _CACHE:
        _CACHE["nc1"] = _split_waits(build_spatial())
        _CACHE["nc2"] = _split_waits(build_temporal())
    nc1, nc2 = _CACHE["nc1"], _CACHE["nc2"]

    # ---- phase 1: spatial over (b t) ----
    xs = np.ascontiguousarray(
        x.transpose(0, 2, 1, 3, 4).reshape(B * T, C, L1)
    )
    common1 = dict(wq=wq1, wk=wk1, wv=wv1, wo=wo1,
                   bq=bq1, bv=bv1, bo=bo1,
                   gmask=gmask1, bmask=bmask, ident=ident)
    in_maps1 = [
        dict(xs=np.ascontiguousarray(xs[i * NS1:(i + 1) * NS1])
             .astype(ml_dtypes.bfloat16), **common1)
        for i in range(N_CORES)
    ]
    _CACHE["in_maps1"] = in_maps1
    r1 = run_bass_kernel_spmd(nc1, in_maps1, core_ids=list(range(N_CORES)),
                              **_CACHE.get("run_kwargs", {}))
    ys = np.concatenate([r1.results[i]["ys"] for i in range(N_CORES)], axis=0)
    _CACHE["last_r1"] = r1

    # ---- phase 2: temporal over (b h w) ----
    x2 = ys.reshape(B, T, C, H, W).transpose(0, 3, 4, 2, 1)  # (b,h,w,c,t)
    x2 = x2.reshape(B * H * W, C, NT2)
    common2 = dict(wq=wq2, wk=wk2, wv=wv2, wo=wo2,
                   bq=bq2, bv=bv2, bo=bo2,
                   gmask=gmask2.astype(ml_dtypes.bfloat16),
                   bmask=bmask.astype(ml_dtypes.bfloat16),
                   ident=ident, blkmask=blk)
    in_maps2 = []
    for i in range(N_CORES):
        shard = x2[i * NS2:(i + 1) * NS2]          # (256, 512, 16)
        xt = np.ascontiguousarray(shard.transpose(1, 0, 2)).reshape(
            C, NS2 * NT2).astype(ml_dtypes.bfloat16)
        in_maps2.append(dict(xt=xt, **common2))
    _CACHE["in_maps2"] = in_maps2
    r2 = run_bass_kernel_spmd(nc2, in_maps2, core_ids=list(range(N_CORES)),
                              **_CACHE.get("run_kwargs", {}))
    _CACHE["last_r2"] = r2

    out = np.empty((B * H * W, C, NT2), np.float32)
    for i in range(N_CORES):
        yt = r2.results[i]["yt"].reshape(C, NS2, NT2)
        out[i * NS2:(i + 1) * NS2] = yt.transpose(1, 0, 2)
    out = out.reshape(B, H, W, C, NT2).transpose(0, 3, 4, 1, 2)
    return np.ascontiguousarray(out)
